# revision 2
# baseline (speedup 1.0000x reference)
"""Trainium2 Bass kernel for nn_AttModel (masked attention GNN message passing).

Contract: kernel(**inputs) takes the FULL unsharded inputs (x [8,2048,128],
mask [8,2048,2048], Wv/Wk/Wq [128,128], bv/bk/bq [128]) and returns the full
output [8, 2048, 128] float32.

Strategy: data-parallel over batch B=8 across the 8 NeuronCores; the small
weight matrices are replicated. The device kernel runs a fully transposed
dataflow (scores computed as S^T per j-stripe) so no [N,N] transpose is ever
done on device; the host pre-transposes x, mask and the weights (pure layout
marshaling) and post-normalizes/transposes the returned outT/rowsum.

Per core (batch element b):
  qT/kT = relu(W x^T + b) as [h, n] bf16 (fp32r projections, fp32 PSUM)
  v     = relu(x W^T + b) as [j, h] bf16 (via PE transpose of vT)
  for i-chunk (1024) and j-stripe (128):
    sT = kT_j^T @ qT_chunk      (PE, bf16, PSUM fp32)
    eT = exp(sT)                (ACT, PSUM -> SBUF bf16)
    pT = eT * maskT_tile        (DVE, bf16; maskT passed as bf16 from host)
    outT_chunk   += v_j^T @ pT  (PE, accumulated in PSUM over stripes)
    rowsum_chunk += 1^T @ pT    (PE, M=1 matmul)
  Host: out_b = (outT / rowsum)^T
"""

from contextlib import ExitStack

import numpy as np
import ml_dtypes

import concourse.bass as bass
import concourse.bacc as bacc
import concourse.tile as tile
from concourse import mybir
from concourse import bass_utils

B = 8
P = 128
N = 2048
HID = 128
DIN = 128
NJ = N // P      # 16 j-stripes
ICH = 1024       # i-chunk width
NCH = N // ICH   # 2 i-chunks

f32 = mybir.dt.float32
f32r = mybir.dt.float32r
bf = mybir.dt.bfloat16
AF = mybir.ActivationFunctionType
ALU = mybir.AluOpType

_NC_CACHE = {}


def _attention_tile_kernel(ctx, tc, outT, rowsum, xT, maskT,
                           WvT, bv, WkT, bk, WqT, bq, identb):
    nc = tc.nc

    consts = ctx.enter_context(tc.tile_pool(name="consts", bufs=1))
    big = ctx.enter_context(tc.tile_pool(name="big", bufs=1))

    idb = consts.tile([P, P], bf)
    nc.sync.dma_start(out=idb, in_=identb)
    ones_col = consts.tile([P, 1], bf)
    nc.vector.memset(ones_col, 1.0)

    xT_sb = big.tile([P, N], f32r)      # [d, n]
    nc.sync.dma_start(out=xT_sb, in_=xT)
    # per-chunk tiles so downstream matmuls get fine-grained dependencies
    qTc = [big.tile([P, 512], bf, name=f"qT{c}") for c in range(4)]
    kTc = [big.tile([P, 512], bf, name=f"kT{c}") for c in range(4)]
    vNs = [big.tile([P, P], bf, name=f"vN{j}") for j in range(NJ)]

    biases = {}
    for nm, bsrc in (("q", bq), ("k", bk), ("v", bv)):
        bt = consts.tile([P, 1], f32, name=f"bias_{nm}")
        nc.sync.dma_start(out=bt, in_=bsrc)
        biases[nm] = bt

    # setup: projections (weights arrive pre-transposed [d, h])
    with tc.tile_pool(name="setup", bufs=3) as sp, \
         tc.tile_pool(name="setup_ps", bufs=2, space="PSUM") as sps:
        wTs = {}
        for nm, W in (("q", WqT), ("k", WkT), ("v", WvT)):
            wT = sp.tile([P, P], f32r, tag=f"wT_{nm}", name=f"wT_{nm}")
            nc.sync.dma_start(out=wT, in_=W)
            wTs[nm] = wT

        for nm, dest in (("q", qTc), ("k", kTc)):
            for c in range(4):
                pr_ps = sps.tile([P, 512], f32, tag="proj", name=f"proj_{nm}{c}")
                nc.tensor.matmul(pr_ps, lhsT=wTs[nm],
                                 rhs=xT_sb[:, c * 512:(c + 1) * 512],
                                 start=True, stop=True)
                nc.scalar.activation(out=dest[c], in_=pr_ps,
                                     func=AF.Relu, bias=biases[nm], scale=1.0)

        vT = sp.tile([P, N], bf, tag="vT")
        for c in range(4):
            pr_ps = sps.tile([P, 512], f32, tag="proj", name=f"proj_v{c}")
            nc.tensor.matmul(pr_ps, lhsT=wTs["v"],
                             rhs=xT_sb[:, c * 512:(c + 1) * 512],
                             start=True, stop=True)
            nc.scalar.activation(out=vT[:, c * 512:(c + 1) * 512], in_=pr_ps,
                                 func=AF.Relu, bias=biases["v"], scale=1.0)
        for jt in range(NJ):
            v_ps = sps.tile([P, P], bf, tag="vps")
            nc.tensor.transpose(v_ps, vT[:, jt * P:(jt + 1) * P], idb)
            nc.vector.tensor_copy(out=vNs[jt], in_=v_ps)

    # main loop: i-chunks x j-stripes, all in transposed score space
    mask_pool = ctx.enter_context(tc.tile_pool(name="maskp", bufs=8))
    e_pool = ctx.enter_context(tc.tile_pool(name="ep", bufs=4))
    pt_pool = ctx.enter_context(tc.tile_pool(name="ptp", bufs=4))
    out_sb_pool = ctx.enter_context(tc.tile_pool(name="outsbp", bufs=2))
    norm_pool = ctx.enter_context(tc.tile_pool(name="normp", bufs=2))
    s_psum = ctx.enter_context(tc.tile_pool(name="spsum", bufs=2, space="PSUM"))
    o_psum = ctx.enter_context(tc.tile_pool(name="opsum", bufs=1, space="PSUM"))
    r_psum = ctx.enter_context(tc.tile_pool(name="rpsum", bufs=1, space="PSUM"))

    for c in range(NCH):
        i0 = c * ICH
        o_ps = o_psum.tile([P, ICH], f32, tag="o")
        r_ps = r_psum.tile([1, ICH], f32, tag="r")
        for jt in range(NJ):
            mask_t = mask_pool.tile([P, ICH], bf, tag="mask")
            nc.sync.dma_start(out=mask_t,
                              in_=maskT[jt * P:(jt + 1) * P, i0:i0 + ICH])
            s_ps = s_psum.tile([P, ICH], f32, tag="s")
            for cc in range(2):
                icol = i0 + cc * 512
                nc.tensor.matmul(
                    s_ps[:, cc * 512:(cc + 1) * 512],
                    lhsT=kTc[jt // 4][:, (jt % 4) * P:(jt % 4 + 1) * P],
                    rhs=qTc[icol // 512],
                    start=True, stop=True)
            e_t = e_pool.tile([P, ICH], bf, tag="e")
            nc.scalar.activation(out=e_t, in_=s_ps, func=AF.Exp)
            p_t = pt_pool.tile([P, ICH], bf, tag="pt")
            nc.vector.tensor_tensor(out=p_t, in0=e_t, in1=mask_t, op=ALU.mult)
            for cc in range(2):
                nc.tensor.matmul(o_ps[:, cc * 512:(cc + 1) * 512],
                                 lhsT=vNs[jt],
                                 rhs=p_t[:, cc * 512:(cc + 1) * 512],
                                 start=(jt == 0), stop=(jt == NJ - 1))
            for cc in range(2):
                nc.tensor.matmul(r_ps[:, cc * 512:(cc + 1) * 512],
                                 lhsT=ones_col,
                                 rhs=p_t[:, cc * 512:(cc + 1) * 512],
                                 start=(jt == 0), stop=(jt == NJ - 1))

        rs_sb = norm_pool.tile([1, ICH], f32, tag="rs")
        nc.scalar.activation(out=rs_sb, in_=r_ps, func=AF.Copy)
        nc.sync.dma_start(out=rowsum[:, i0:i0 + ICH], in_=rs_sb)
        out_sb = out_sb_pool.tile([P, ICH], f32, tag="osb")
        nc.vector.tensor_copy(out=out_sb, in_=o_ps)
        nc.sync.dma_start(out=outT[:, i0:i0 + ICH], in_=out_sb)


def _build_nc():
    if "nc" in _NC_CACHE:
        return _NC_CACHE["nc"]
    nc = bacc.Bacc("TRN2", target_bir_lowering=False, debug=False, num_devices=B)
    xT = nc.dram_tensor("xT", [DIN, N], f32r, kind="ExternalInput").ap()
    maskT = nc.dram_tensor("maskT", [N, N], bf, kind="ExternalInput").ap()
    WvT = nc.dram_tensor("WvT", [DIN, HID], f32r, kind="ExternalInput").ap()
    bv = nc.dram_tensor("bv", [HID], f32, kind="ExternalInput").ap()
    WkT = nc.dram_tensor("WkT", [DIN, HID], f32r, kind="ExternalInput").ap()
    bk = nc.dram_tensor("bk", [HID], f32, kind="ExternalInput").ap()
    WqT = nc.dram_tensor("WqT", [DIN, HID], f32r, kind="ExternalInput").ap()
    bq = nc.dram_tensor("bq", [HID], f32, kind="ExternalInput").ap()
    identb = nc.dram_tensor("identb", [P, P], bf, kind="ExternalInput").ap()
    outT = nc.dram_tensor("outT", [HID, N], f32, kind="ExternalOutput").ap()
    rowsum = nc.dram_tensor("rowsum", [1, N], f32, kind="ExternalOutput").ap()

    with tile.TileContext(nc) as tc:
        with ExitStack() as ctx:
            _attention_tile_kernel(ctx, tc, outT, rowsum, xT, maskT,
                                   WvT, bv, WkT, bk, WqT, bq, identb)
    nc.compile()
    _NC_CACHE["nc"] = nc
    return nc


def make_in_maps(x, mask, Wv, bv, Wk, bk, Wq, bq):
    x = np.asarray(x, dtype=np.float32)
    mask = np.asarray(mask, dtype=np.float32)
    Wv = np.asarray(Wv, dtype=np.float32)
    bv = np.asarray(bv, dtype=np.float32)
    Wk = np.asarray(Wk, dtype=np.float32)
    bk = np.asarray(bk, dtype=np.float32)
    Wq = np.asarray(Wq, dtype=np.float32)
    bq = np.asarray(bq, dtype=np.float32)

    identb = np.eye(P, dtype=ml_dtypes.bfloat16)
    WvT = np.ascontiguousarray(Wv.T)
    WkT = np.ascontiguousarray(Wk.T)
    WqT = np.ascontiguousarray(Wq.T)
    in_maps = []
    for c in range(B):
        in_maps.append({
            "xT": np.ascontiguousarray(x[c].T),
            "maskT": np.ascontiguousarray(mask[c].T.astype(ml_dtypes.bfloat16)),
            "WvT": WvT, "bv": bv, "WkT": WkT, "bk": bk, "WqT": WqT, "bq": bq,
            "identb": identb,
        })
    return in_maps


def postprocess(res):
    out = np.empty((B, N, HID), dtype=np.float32)
    for c in range(B):
        outT = res.results[c]["outT"]
        rowsum = res.results[c]["rowsum"]
        rowsum = np.where(rowsum == 0.0, 1.0, rowsum)
        out[c] = (outT / rowsum).T
    return out


def kernel(x, mask, Wv, bv, Wk, bk, Wq, bq):
    nc = _build_nc()
    in_maps = make_in_maps(x, mask, Wv, bv, Wk, bk, Wq, bq)
    res = bass_utils.run_bass_kernel_spmd(nc, in_maps, core_ids=list(range(B)),
                                          trace=False)
    return postprocess(res)



# revision 10
# speedup vs baseline: 1.1358x; 1.1358x over previous
"""Trainium2 Bass kernel for nn_AttModel (masked attention GNN message passing).

Contract: kernel(**inputs) takes the FULL unsharded inputs (x [8,2048,128],
mask [8,2048,2048], Wv/Wk/Wq [128,128], bv/bk/bq [128]) and returns the full
output [8, 2048, 128] float32.

Strategy: data-parallel over batch B=8 across the 8 NeuronCores; the small
weight matrices are replicated. The device kernel runs a fully transposed
dataflow (scores computed as S^T per j-stripe) so no [N,N] transpose is ever
done on device; the host pre-transposes x, mask and the weights (pure layout
marshaling) and post-normalizes/transposes the returned outT/rowsum.

Per core (batch element b), all bf16 data path, f32 PSUM accumulation:
  qT/kT = relu(W x^T + b) as [h, n] bf16
  v     = relu(x W^T + b) computed directly in natural [j, h] orientation:
          bias broadcast via a K=1 ones-row matmul into PSUM, then x-slice
          matmuls accumulate, relu on DVE (no transposes anywhere)
  for i-chunk (1024) and j-stripe (128):
    sT = kT_j^T @ qT_chunk      (PE, bf16, PSUM f32)
    eT = exp(sT)                (ACT, PSUM -> SBUF bf16)
    pT = eT * maskT_tile        (DVE, bf16)
    outT_chunk   += v_j^T @ pT  (PE, accumulated in PSUM over stripes)
    every odd stripe: p01 = pT_prev + pT (DVE), rowsum += 1^T @ p01 (PE)
  Host: out_b = (outT / rowsum)^T

Scheduling: all 32 mask tiles are prefetched at kernel start (sync DMA ring);
v-transposes + output stores ride the scalar DMA ring (separate HWDGE FIFO).
s-matmuls run two stripes ahead of the out-matmuls; the exp table is
pre-warmed with a dummy activation; late projections (k1-3, v1-3, q2-3) are
deferred into the first loop bodies, sharing the s PSUM ring.
"""

from contextlib import ExitStack

import numpy as np
import ml_dtypes

import concourse.bass as bass
import concourse.bacc as bacc
import concourse.tile as tile
from concourse import mybir
from concourse import bass_utils

B = 8
P = 128
N = 2048
HID = 128
DIN = 128
NJ = N // P      # 16 j-stripes
ICH = 1024       # i-chunk width
NCH = N // ICH   # 2 i-chunks
NT = NCH * NJ    # 32 global stripes

f32 = mybir.dt.float32
bf = mybir.dt.bfloat16
AF = mybir.ActivationFunctionType
ALU = mybir.AluOpType

_NC_CACHE = {}


def _attention_tile_kernel(ctx, tc, outT, rowsum, xT, maskT,
                           WvT, bv, WkT, bk, WqT, bq):
    nc = tc.nc

    consts = ctx.enter_context(tc.tile_pool(name="consts", bufs=1))
    big = ctx.enter_context(tc.tile_pool(name="big", bufs=1))
    ps = ctx.enter_context(tc.tile_pool(name="ps", bufs=1, space="PSUM"))
    e_pool = ctx.enter_context(tc.tile_pool(name="ep", bufs=6))
    p_pool = ctx.enter_context(tc.tile_pool(name="pp", bufs=4))
    pp_pool = ctx.enter_context(tc.tile_pool(name="pairp", bufs=2))
    drain_pool = ctx.enter_context(tc.tile_pool(name="drainp", bufs=2))

    # pre-warm the exp table set while DMAs are in flight
    warm_in = consts.tile([P, 1], f32)
    nc.vector.memset(warm_in, 0.0)
    warm_out = consts.tile([P, 1], bf)
    nc.scalar.activation(out=warm_out, in_=warm_in, func=AF.Exp)

    ones_col = consts.tile([P, 1], bf)
    nc.vector.memset(ones_col, 1.0)
    ones_row = consts.tile([1, P], bf)
    nc.vector.memset(ones_row, 1.0)

    bvR_t = consts.tile([1, 512], bf)
    nc.sync.dma_start(out=bvR_t, in_=bv)
    biases = {}
    for nm, bsrc in (("q", bq), ("k", bk)):
        bt = consts.tile([P, 1], f32, name=f"bias_{nm}")
        nc.sync.dma_start(out=bt, in_=bsrc)
        biases[nm] = bt
    wts = {}
    for nm, wsrc in (("q", WqT), ("k", WkT), ("v", WvT)):
        wt = consts.tile([P, P], bf, name=f"w_{nm}")
        nc.sync.dma_start(out=wt, in_=wsrc)
        wts[nm] = wt

    # x arrives transposed [d, n] bf16, in 4 column chunks
    xc = []
    for c in range(4):
        xt = big.tile([P, 512], bf, name=f"xc{c}")
        nc.sync.dma_start(out=xt, in_=xT[:, c * 512:(c + 1) * 512])
        xc.append(xt)

    # prefetch ALL mask tiles (sync ring streams them behind x)
    masks = []
    for t in range(NT):
        c, j = t // NJ, t % NJ
        mt = big.tile([P, ICH], bf, name=f"mask{t}")
        nc.sync.dma_start(
            out=mt, in_=maskT[j * P:(j + 1) * P, c * ICH:(c + 1) * ICH])
        masks.append(mt)

    qT = [big.tile([P, ICH], bf, name=f"qT{c}") for c in range(NCH)]
    kT = big.tile([P, N], bf)
    # v in natural [j, h] orientation, packed 4 stripes per tile
    vb = [big.tile([P, 512], bf, name=f"vb{c}") for c in range(4)]

    def vN(j):
        return vb[j // 4][:, (j % 4) * P:(j % 4 + 1) * P]

    def proj(nm, c):
        """Project chunk c (columns 512c:512c+512) of q/k/v; relu into SBUF."""
        pt = ps.tile([P, 512], f32, tag="s", bufs=2, name=f"proj_{nm}{c}")
        if nm == "v":
            # per stripe: bias broadcast (K=1 ones matmul) + x @ WvT on top
            for jj in range(4):
                sub = pt[:, jj * P:(jj + 1) * P]
                nc.tensor.matmul(sub, lhsT=ones_row,
                                 rhs=bvR_t[:, jj * P:(jj + 1) * P],
                                 start=True, stop=False)
                nc.tensor.matmul(sub, lhsT=xc[c][:, jj * P:(jj + 1) * P],
                                 rhs=wts["v"], start=False, stop=True)
            nc.vector.tensor_scalar(vb[c], pt, 0.0, None, op0=ALU.max)
            return
        nc.tensor.matmul(pt, lhsT=wts[nm], rhs=xc[c], start=True, stop=True)
        if nm == "k":
            nc.scalar.activation(out=kT[:, c * 512:(c + 1) * 512], in_=pt,
                                 func=AF.Relu, bias=biases["k"], scale=1.0)
        else:
            dest = qT[c // 2][:, (c % 2) * 512:(c % 2 + 1) * 512]
            if c < 2:
                nc.scalar.activation(out=dest, in_=pt, func=AF.Relu,
                                     bias=biases["q"], scale=1.0)
            else:
                nc.vector.tensor_scalar(dest, pt, biases["q"], 0.0,
                                        op0=ALU.add, op1=ALU.max)

    # prologue projections: just enough for the first two stripes
    proj("k", 0)
    proj("q", 0)
    proj("q", 1)
    proj("v", 0)

    s_tiles = {}
    p_tiles = {}

    def emit_s(t):
        c, j = t // NJ, t % NJ
        sp = ps.tile([P, ICH], f32, tag="s", bufs=2, name=f"s{t}")
        for cc in range(2):
            nc.tensor.matmul(sp[:, cc * 512:(cc + 1) * 512],
                             lhsT=kT[:, j * P:(j + 1) * P],
                             rhs=qT[c][:, cc * 512:(cc + 1) * 512],
                             start=True, stop=True)
        e_t = e_pool.tile([P, ICH], bf, tag="e", name=f"e{t}")
        nc.scalar.activation(out=e_t, in_=sp, func=AF.Exp)
        p_t = p_pool.tile([P, ICH], bf, tag="p", name=f"p{t}")
        nc.vector.tensor_tensor(out=p_t, in0=e_t, in1=masks[t], op=ALU.mult)
        p_tiles[t] = p_t

    emit_s(0)
    emit_s(1)

    # late projections, folded into the first loop bodies (2 per body)
    deferred = [("v", 1), ("k", 1), ("v", 2), ("k", 2),
                ("v", 3), ("k", 3), ("q", 2), ("q", 3)]

    o_ps = None
    r_ps = None
    for t in range(NT):
        c, j = t // NJ, t % NJ
        i0 = c * ICH

        if j == 0:
            o_ps = ps.tile([P, ICH], f32, tag="o", bufs=1, name=f"o{c}")
        if t < 4:
            proj(*deferred[2 * t])
            proj(*deferred[2 * t + 1])
        if t + 2 < NT:
            emit_s(t + 2)

        p_t = p_tiles[t]
        for cc in range(2):
            nc.tensor.matmul(o_ps[:, cc * 512:(cc + 1) * 512],
                             lhsT=vN(j), rhs=p_t[:, cc * 512:(cc + 1) * 512],
                             start=(j == 0), stop=(j == NJ - 1))
        if j % 2 == 1:
            p01 = pp_pool.tile([P, ICH], bf, tag="p01", name=f"p01_{t}")
            nc.vector.tensor_tensor(out=p01, in0=p_tiles[t - 1], in1=p_t,
                                    op=ALU.add)
            if j == 1:
                r_ps = ps.tile([1, ICH], f32, tag="r", bufs=1, name=f"r{c}")
            for cc in range(2):
                nc.tensor.matmul(r_ps[:, cc * 512:(cc + 1) * 512],
                                 lhsT=ones_col,
                                 rhs=p01[:, cc * 512:(cc + 1) * 512],
                                 start=(j == 1), stop=(j == NJ - 1))
            del p_tiles[t - 1], p_tiles[t]

        if j == NJ - 1:
            # drain this chunk: PSUM -> SBUF (DVE) -> HBM (scalar ring)
            rs_sb = drain_pool.tile([1, ICH], f32, tag="rs", name=f"rs{c}")
            nc.vector.tensor_copy(out=rs_sb, in_=r_ps)
            nc.scalar.dma_start(out=rowsum[:, i0:i0 + ICH], in_=rs_sb)
            for cc in range(2):
                osb = drain_pool.tile([P, 512], f32, tag="osb",
                                      name=f"osb{c}_{cc}")
                nc.vector.tensor_copy(out=osb,
                                      in_=o_ps[:, cc * 512:(cc + 1) * 512])
                nc.scalar.dma_start(
                    out=outT[:, i0 + cc * 512:i0 + (cc + 1) * 512], in_=osb)


def _build_nc():
    if "nc" in _NC_CACHE:
        return _NC_CACHE["nc"]
    nc = bacc.Bacc("TRN2", target_bir_lowering=False, debug=False, num_devices=B)
    xT = nc.dram_tensor("xT", [DIN, N], bf, kind="ExternalInput").ap()
    maskT = nc.dram_tensor("maskT", [N, N], bf, kind="ExternalInput").ap()
    WvT = nc.dram_tensor("WvT", [DIN, HID], bf, kind="ExternalInput").ap()
    bv = nc.dram_tensor("bv", [1, 512], bf, kind="ExternalInput").ap()
    WkT = nc.dram_tensor("WkT", [DIN, HID], bf, kind="ExternalInput").ap()
    bk = nc.dram_tensor("bk", [HID], f32, kind="ExternalInput").ap()
    WqT = nc.dram_tensor("WqT", [DIN, HID], bf, kind="ExternalInput").ap()
    bq = nc.dram_tensor("bq", [HID], f32, kind="ExternalInput").ap()
    outT = nc.dram_tensor("outT", [HID, N], f32, kind="ExternalOutput").ap()
    rowsum = nc.dram_tensor("rowsum", [1, N], f32, kind="ExternalOutput").ap()

    with tile.TileContext(nc) as tc:
        with ExitStack() as ctx:
            _attention_tile_kernel(ctx, tc, outT, rowsum, xT, maskT,
                                   WvT, bv, WkT, bk, WqT, bq)
    nc.compile()
    _NC_CACHE["nc"] = nc
    return nc


def make_in_maps(x, mask, Wv, bv, Wk, bk, Wq, bq):
    x = np.asarray(x, dtype=np.float32)
    mask = np.asarray(mask, dtype=np.float32)
    Wv = np.asarray(Wv, dtype=np.float32)
    bv = np.asarray(bv, dtype=np.float32)
    Wk = np.asarray(Wk, dtype=np.float32)
    bk = np.asarray(bk, dtype=np.float32)
    Wq = np.asarray(Wq, dtype=np.float32)
    bq = np.asarray(bq, dtype=np.float32)

    WvT = np.ascontiguousarray(Wv.T.astype(ml_dtypes.bfloat16))
    WkT = np.ascontiguousarray(Wk.T.astype(ml_dtypes.bfloat16))
    WqT = np.ascontiguousarray(Wq.T.astype(ml_dtypes.bfloat16))
    bvR = np.ascontiguousarray(np.tile(bv, 4)[None, :].astype(ml_dtypes.bfloat16))
    in_maps = []
    for c in range(B):
        in_maps.append({
            "xT": np.ascontiguousarray(x[c].T.astype(ml_dtypes.bfloat16)),
            "maskT": np.ascontiguousarray(mask[c].T.astype(ml_dtypes.bfloat16)),
            "WvT": WvT, "bv": bvR, "WkT": WkT, "bk": bk, "WqT": WqT, "bq": bq,
        })
    return in_maps


def postprocess(res):
    out = np.empty((B, N, HID), dtype=np.float32)
    for c in range(B):
        outT = res.results[c]["outT"]
        rowsum = res.results[c]["rowsum"]
        rowsum = np.where(rowsum == 0.0, 1.0, rowsum)
        out[c] = (outT / rowsum).T
    return out


def kernel(x, mask, Wv, bv, Wk, bk, Wq, bq):
    nc = _build_nc()
    in_maps = make_in_maps(x, mask, Wv, bv, Wk, bk, Wq, bq)
    res = bass_utils.run_bass_kernel_spmd(nc, in_maps, core_ids=list(range(B)),
                                          trace=False)
    return postprocess(res)
